# revision 1
# baseline (speedup 1.0000x reference)
"""Trainium2 Bass kernel for nn_Encoder (GNN message passing / SAT encoder).

Strategy (8 NeuronCores, data-parallel over the variable dimension V):
  - Host precomputes the literal-rotation permutation (prepare_variables) and
    folds it, together with lit_neg / lit_valid / clause_valid, into static
    int16 gather-index tables per core.
  - Each core keeps a per-core *compacted* embedding table in DRAM holding
    only the (plain, negated, false) rows its literals reference, so the fast
    int16 `dma_gather` path can be used (table < 32767 rows).
  - Per iteration: gather literal embeddings -> PE-transpose -> clause combine
    (matmul + sigmoid + l2norm) -> scatter c_emb through DRAM -> gather into
    per-variable layout -> variable combine -> has_clause select.
  - Between iterations: shards are AllGathered; each core re-gathers its
    compacted table (plain + negated sections) from the global tables.
"""

import math
from contextlib import ExitStack

import numpy as np

import concourse.bass as bass
import concourse.bacc as bacc
import concourse.mybir as mybir
import concourse.tile as tile
from concourse.bass import Bass
from concourse.bass_utils import run_bass_kernel_spmd
from concourse.masks import make_identity
from concourse.library_config import mlp as _mlp_lib

F32 = mybir.dt.float32
I16 = mybir.dt.int16
AF = mybir.ActivationFunctionType
ALU = mybir.AluOpType

NCORES = 8
P = 128


def _l2norm(x):
    n = np.sqrt(np.sum(x * x, axis=-1, keepdims=True))
    return x / np.maximum(n, 1e-12)


def _wrap_idx(flat):
    """dma_gather idx layout: idx j -> [j%16, j//16] int16, replicated x8."""
    n = len(flat)
    assert n % 16 == 0
    w = np.asarray(flat, np.int16).reshape(-1, 16).T.copy()  # [16, n/16]
    return np.tile(w, (8, 1))  # [128, n/16]


def _ceil_to(x, m):
    return ((x + m - 1) // m) * m


class HostPlan:
    """All host-side index math; static across both iterations."""

    def __init__(self, V, G, C, M, E, clause_lits, lit_neg, lit_valid,
                 clause_valid):
        self.V, self.G, self.C, self.M, self.E = V, G, C, M, E
        VS = V // NCORES
        self.VS = VS
        self.NG = _ceil_to(VS, P) // P            # 128-var groups per core
        self.VSP = self.NG * P                    # padded shard vars

        # rotation permutation (mirror of reference.prepare_variables)
        var_ids = np.arange(V)[:, None, None]
        ind = np.argmax((clause_lits == var_ids) & (lit_valid > 0), axis=-1)
        p = np.arange(M)[None, None, :]
        ind_b = ind[:, :, None]
        gidx = np.where(p == 0, ind_b, np.where(p - 1 < ind_b, p - 1, p))
        self.psrc = np.take_along_axis(clause_lits, gidx, 2)   # [V,C,M]
        self.pneg = np.take_along_axis(lit_neg, gidx, 2)
        self.pval = np.take_along_axis(lit_valid, gidx, 2)
        self.cval = clause_valid > 0                           # [V,C]
        self.has_clause = self.cval.any(1)                     # [V]

        # per-core valid (v, c) rows, v-major
        self.rows = []     # list of (nr, rows_v(local), rows_c)
        self.plain_list = []
        self.neg_list = []
        for k in range(NCORES):
            vlo = k * VS
            m = self.cval[vlo:vlo + VS]
            rv, rc = np.nonzero(m)                 # v-major order
            self.rows.append((len(rv), rv, rc))
            src = self.psrc[vlo + rv, rc]          # [nr, M]
            neg = self.pneg[vlo + rv, rc] > 0
            val = self.pval[vlo + rv, rc] > 0
            self.plain_list.append(np.unique(src[val & ~neg]))
            self.neg_list.append(np.unique(src[val & neg]))

        # global padded sizes (same NEFF on all cores)
        maxrows = max(r[0] for r in self.rows)
        self.NTILES = _ceil_to(_ceil_to(maxrows, P) // P, 4)   # vc tiles
        self.NR = self.NTILES * P
        self.NP = _ceil_to(max(len(x) for x in self.plain_list), P)
        self.NN = _ceil_to(max(len(x) for x in self.neg_list), P)
        self.GP = self.NP // P
        self.GN = self.NN // P
        self.FALSE_POS = self.NP + self.NN
        self.TC = self.NP + self.NN + 1            # compact table rows
        assert self.TC < 32767, self.TC
        self.ZROW = self.NTILES                    # zero row in c_emb dram
        self.CEMB_ROWS = P * (self.NTILES + 1)
        assert self.CEMB_ROWS < 32767

        # global (allgathered) table row of var v, raw SBUF-major shard layout:
        # rank k rows [k*VSP, (k+1)*VSP), local var l at row (l%128)*NG + l//128
        def gtab_row(v):
            k = v // VS
            l = v - k * VS
            return k * self.VSP + (l % P) * self.NG + l // P
        vv = np.arange(V)
        self.gtab_row = (vv // VS) * self.VSP + ((vv % VS) % P) * self.NG \
            + (vv % VS) // P
        self.GT_ROWS = NCORES * self.VSP
        assert self.GT_ROWS < 32767

        # per-core index tensors
        self.idx_main = []   # [128, NR*M/16] i16
        self.idx_prod_p = []  # [128, NP/16]
        self.idx_prod_n = []
        self.idx_x2 = []     # [128, VSP*C/16]
        self.noclause = []   # [128, NG] f32 (1.0 where var has NO clause)
        for k in range(NCORES):
            vlo = k * VS
            nr, rv, rc = self.rows[k]
            # compact positions
            pl, ng = self.plain_list[k], self.neg_list[k]
            ppos = np.zeros(V, np.int64)
            # plain row j of gather -> compact row (j%128)*GP + j//128
            j = np.arange(len(pl))
            ppos[pl] = (j % P) * self.GP + j // P
            npos = np.zeros(V, np.int64)
            j = np.arange(len(ng))
            npos[ng] = self.NP + (j % P) * self.GN + j // P

            # main gather: tile t, partition p -> row r = t*128+p
            # flat gather j within call over tiles [t0,t0+tb): g=(t-t0)*M+m
            src = self.psrc[vlo + rv, rc]
            neg = self.pneg[vlo + rv, rc] > 0
            val = self.pval[vlo + rv, rc] > 0
            slot = np.where(~val, self.FALSE_POS,
                            np.where(neg, npos[src], ppos[src]))  # [nr, M]
            full = np.full((self.NR, self.M), self.FALSE_POS, np.int64)
            full[:nr] = slot
            # gather flat order per super-call handled at call sites; build
            # one wrapped array per call of TB tiles
            self.idx_main.append(full)

            # production gathers: j -> global table row of list[j]
            fp = np.zeros(self.NP, np.int64)
            fp[:len(pl)] = self.gtab_row[pl]
            fn = np.zeros(self.NN, np.int64)
            fn[:len(ng)] = self.gtab_row[ng]
            self.idx_prod_p.append(_wrap_idx(fp))
            self.idx_prod_n.append(_wrap_idx(fn))

            # x2 gather: group gi, flat j = c*128 + p -> (var gi*128+p, c)
            # c_emb compact row of (v,c)-row r: (r%128)*NTILES + r//128
            rowid = np.full((VS, self.C), self.ZROW, np.int64)
            r = np.arange(nr)
            rowid[rv, rc] = (r % P) * (self.NTILES + 1) + r // P
            x2 = np.full((self.NG, self.C, P), self.ZROW, np.int64)
            rowid_pad = np.full((self.VSP, self.C), self.ZROW, np.int64)
            rowid_pad[:VS] = rowid
            x2 = rowid_pad.reshape(self.NG, P, self.C).transpose(0, 2, 1)
            self.idx_x2.append(_wrap_idx(x2.reshape(-1)))

            hc = np.zeros((self.VSP,), np.int8)
            hc[:VS] = (~self.has_clause[vlo:vlo + VS]).astype(np.int8)
            self.noclause.append(hc.reshape(self.NG, P).T.copy())  # [128, NG]

    def wrap_main_idx(self, k, TB):
        """Wrapped main-gather idx for core k, concatenated per super-call of
        TB tiles (last call may be shorter)."""
        full = self.idx_main[k]  # [NR, M]
        chunks = []
        t0 = 0
        while t0 < self.NTILES:
            tb = min(TB, self.NTILES - t0)
            n = tb * P * self.M
            flat = np.empty(n, np.int64)
            # j = g*128 + p, g = (t-t0)*M + m
            blk = full[t0 * P:(t0 + tb) * P].reshape(tb, P, self.M)
            flat = blk.transpose(0, 2, 1).reshape(-1)  # (t, m, p)
            chunks.append(_wrap_idx(flat))
            t0 += tb
        return np.concatenate(chunks, axis=1)  # [128, NR*M/16]


def build_program(plan: HostPlan, TB=8, GB=4, for_sim=False, stages=99):
    """Build the SPMD bass program. Returns (nc, input_names)."""
    V, C, M, E = plan.V, plan.C, plan.M, plan.E
    NT, NG = plan.NTILES, plan.NG
    KC = (M * E) // P       # vc contraction chunks (3)
    KC2 = (C * E) // P      # cc contraction chunks (4)
    nc = bacc.Bacc("TRN2", num_devices=NCORES)

    # ---- external inputs (per-core data) ----
    ein = lambda name, shape, dt=F32: nc.dram_tensor(
        name, shape, dt, kind="ExternalInput")
    t_compact1 = ein("compact1", [plan.TC, E])
    t_idx_main = ein("idx_main", [P, plan.NR * M // 16], I16)
    t_idx_pp = ein("idx_prod_p", [P, plan.NP // 16], I16)
    t_idx_pn = ein("idx_prod_n", [P, plan.NN // 16], I16)
    t_idx_x2 = ein("idx_x2", [P, plan.VSP * C // 16], I16)
    t_shard0 = ein("shard0", [P, NG * E])          # raw sbuf-major layout
    t_noclause = ein("noclause", [P, NG], mybir.dt.int8)
    t_wvc = ein("wvc", [P, KC * P])                # vc [W1|W2] chunks
    t_wcc = ein("wcc", [P, KC2 * P])
    t_bvc = ein("bvc", [E, 1])
    t_bvc2 = ein("bvc2", [E, 1])
    t_bcc = ein("bcc", [E, 1])
    t_bcc2 = ein("bcc2", [E, 1])
    t_wneg = ein("wneg_aug", [E + 1, E])           # [W_neg; b_neg]
    t_false = ein("false_row", [1, E])

    t_out = nc.dram_tensor("out_shard", [P, NG * E], F32, kind="ExternalOutput")

    # ---- internal DRAM ----
    t_compact2 = nc.dram_tensor("compact2", [plan.TC, E], F32)
    t_cemb = [nc.dram_tensor(f"cemb{i}", [plan.CEMB_ROWS, E], F32)
              for i in range(2)]
    t_agp_in = nc.dram_tensor("agp_in", [plan.VSP, E], F32)
    t_agn_in = nc.dram_tensor("agn_in", [plan.VSP, E], F32)
    t_ptab = nc.dram_tensor("ptab", [plan.GT_ROWS, E], F32)
    t_ntab = nc.dram_tensor("ntab", [plan.GT_ROWS, E], F32)

    with tile.TileContext(nc) as tc, ExitStack() as ctx:
        const = ctx.enter_context(tc.tile_pool(name="const", bufs=1))
        sb_x = ctx.enter_context(tc.tile_pool(name="sb_x", bufs=2))
        sb_xt = ctx.enter_context(tc.tile_pool(name="sb_xt", bufs=2))
        sb_ep = ctx.enter_context(tc.tile_pool(name="sb_ep", bufs=3))
        sb_big = ctx.enter_context(tc.tile_pool(name="sb_big", bufs=1))
        ps_tp = ctx.enter_context(tc.tile_pool(name="ps_tp", bufs=2, space="PSUM"))
        ps_mm = ctx.enter_context(tc.tile_pool(name="ps_mm", bufs=2, space="PSUM"))
        ps_sm = ctx.enter_context(tc.tile_pool(name="ps_sm", bufs=3, space="PSUM"))


        # constants / persistent state
        ident = const.tile([P, P], F32)
        make_identity(nc, ident)
        wvc = const.tile([P, KC * P], F32)
        nc.sync.dma_start(wvc[:], t_wvc[:])
        wcc = const.tile([P, KC2 * P], F32)
        nc.sync.dma_start(wcc[:], t_wcc[:])
        bvc = const.tile([E, 1], F32)
        nc.sync.dma_start(bvc[:], t_bvc[:])
        bvc2 = const.tile([E, 1], F32)
        nc.sync.dma_start(bvc2[:], t_bvc2[:])
        bcc = const.tile([E, 1], F32)
        nc.sync.dma_start(bcc[:], t_bcc[:])
        bcc2 = const.tile([E, 1], F32)
        nc.sync.dma_start(bcc2[:], t_bcc2[:])
        wneg = const.tile([E + 1, E], F32)
        nc.sync.dma_start(wneg[:], t_wneg[:])
        false_sb = const.tile([1, E], F32)
        nc.sync.dma_start(false_sb[:], t_false[:])
        nocl = const.tile([P, NG], mybir.dt.int8)
        nc.sync.dma_start(nocl[:], t_noclause[:])
        idxm = const.tile([P, plan.NR * M // 16], I16)
        nc.sync.dma_start(idxm[:], t_idx_main[:])
        idxpp = const.tile([P, plan.NP // 16], I16)
        nc.sync.dma_start(idxpp[:], t_idx_pp[:])
        idxpn = const.tile([P, plan.NN // 16], I16)
        nc.sync.dma_start(idxpn[:], t_idx_pn[:])
        idxx2 = const.tile([P, plan.VSP * C // 16], I16)
        nc.sync.dma_start(idxx2[:], t_idx_x2[:])

        shard = [const.tile([P, NG * E], F32, tag=f"shard{i}", name=f"shard{i}")
                 for i in range(2)]
        nc.sync.dma_start(shard[0][:], t_shard0[:])

        cemb_stage = const.tile([P, (NT + 1) * E], F32)
        nc.vector.memset(cemb_stage[:, NT * E:(NT + 1) * E], 0.0)
        epsv = const.tile([P, 1], F32)
        nc.vector.memset(epsv[:], 1e-24)

        n_iters = 1 if stages < 3 else 2
        for it in range(n_iters):
            ctab = t_compact1 if (it == 0 or stages < 4) else t_compact2
            cemb = t_cemb[it]
            cur, nxt = shard[it % 2], shard[(it + 1) % 2]

            if it == 1 and stages >= 4:
                # exchange: allgather shards, rebuild compact table
                nc.sync.dma_start(t_agp_in[:], cur[:].rearrange("p (g e) -> p g e", e=E))
                nc.sync.dma_start(t_agn_in[:], negstage[:].rearrange("p (g e) -> p g e", e=E))
                nc.gpsimd.collective_compute(
                    "AllGather", ALU.bypass,
                    replica_groups=[list(range(NCORES))],
                    ins=[t_agp_in[:]], outs=[t_ptab[:]])
                nc.gpsimd.collective_compute(
                    "AllGather", ALU.bypass,
                    replica_groups=[list(range(NCORES))],
                    ins=[t_agn_in[:]], outs=[t_ntab[:]])
                bufp = sb_big.tile([P, plan.GP * E], F32, tag="prod")
                nc.gpsimd.dma_gather(
                    bufp[:].rearrange("p (g e) -> p g e", e=E), t_ptab[:],
                    idxpp[:], plan.NP, plan.NP, E, single_packet=False)
                nc.sync.dma_start(t_compact2[0:plan.NP, :], bufp[:])
                bufn = sb_big.tile([P, plan.GN * E], F32, tag="prod")
                nc.gpsimd.dma_gather(
                    bufn[:].rearrange("p (g e) -> p g e", e=E), t_ntab[:],
                    idxpn[:], plan.NN, plan.NN, E, single_packet=False)
                nc.sync.dma_start(t_compact2[plan.NP:plan.NP + plan.NN, :], bufn[:])
                nc.sync.dma_start(
                    t_compact2[plan.FALSE_POS:plan.FALSE_POS + 1, :], false_sb[:])

            # ---- clause stage ----
            xbuf = None
            for grp in range(NT // 4):
                if grp % (TB // 4) == 0:
                    t0 = grp * 4
                    tb = min(TB, NT - t0)
                    xbuf = sb_x.tile([P, TB * M * E], F32, tag="xbuf")
                    n = tb * P * M
                    col0 = t0 * P * M // 16
                    nc.gpsimd.dma_gather(
                        xbuf[:, :tb * M * E].rearrange("p (g e) -> p g e", e=E),
                        ctab[:], idxm[:, col0:col0 + n // 16], n, n, E,
                        single_packet=False)
                xT = [sb_xt.tile([P, 4 * P], F32, tag=f"xT{k}", name=f"xT{k}")
                      for k in range(KC)]
                for tt in range(4):
                    toff = (grp * 4) % TB + tt
                    for k in range(KC):
                        tp = ps_tp.tile([P, P], F32, tag="tp")
                        nc.tensor.transpose(
                            tp[:], xbuf[:, toff * M * E + k * P: toff * M * E + (k + 1) * P],
                            ident[:])
                        eng = nc.scalar if (tt * KC + k) % 2 == 0 else nc.vector
                        if eng is nc.scalar:
                            nc.scalar.copy(xT[k][:, tt * P:(tt + 1) * P], tp[:])
                        else:
                            nc.vector.tensor_copy(xT[k][:, tt * P:(tt + 1) * P], tp[:])
                psY = ps_mm.tile([P, 4 * P], F32, tag="mm")
                for k in range(KC):
                    nc.tensor.matmul(psY[:], lhsT=wvc[:, k * P:(k + 1) * P],
                                     rhs=xT[k][:], start=(k == 0), stop=(k == KC - 1))
                s = sb_ep.tile([E, 4 * P], F32, tag="s")
                nc.scalar.activation(out=s[:], in_=psY[0:E, :], func=AF.Sigmoid,
                                     bias=bvc[:], scale=1.0)
                tt_ = sb_ep.tile([E, 4 * P], F32, tag="t")
                nc.vector.scalar_tensor_tensor(
                    out=tt_[:], in0=psY[E:2 * E, :], scalar=bvc2[:],
                    in1=s[:], op0=ALU.add, op1=ALU.add)
                for q in range(4):
                    tp2 = ps_sm.tile([P, E], F32, tag="sm")
                    nc.tensor.transpose(tp2[:], tt_[:, q * P:(q + 1) * P],
                                        ident[0:E, 0:E])
                    craw = sb_ep.tile([P, E], F32, tag="craw")
                    nc.scalar.copy(craw[:], tp2[:])
                    sq = sb_ep.tile([P, E], F32, tag="sq")
                    ss = sb_ep.tile([P, 1], F32, tag="ss")
                    nc.vector.tensor_mul(sq[:], craw[:], craw[:])
                    nc.vector.reduce_sum(ss[:], sq[:], axis=mybir.AxisListType.X)
                    rstd = sb_ep.tile([P, 1], F32, tag="rstd")
                    nc.scalar.activation(out=rstd[:], in_=ss[:], func=AF.Sqrt,
                                         bias=epsv[:])
                    nc.vector.reciprocal(out=rstd[:], in_=rstd[:])
                    tcol = grp * 4 + q
                    nc.vector.tensor_scalar_mul(
                        cemb_stage[:, tcol * E:(tcol + 1) * E], craw[:], rstd[:])
            nc.sync.dma_start(cemb[:],
                              cemb_stage[:].rearrange("p (t e) -> p t e", e=E))

            # ---- variable stage ----
            if stages < 2:
                break
            if it == 0:
                negstage = const.tile([P, NG * E], F32)
            for call in range(math.ceil(NG / GB)):
                g0 = call * GB
                gb = min(GB, NG - g0)
                nx2 = gb * C * P
                x2buf = sb_x.tile([P, GB * C * E], F32, tag="x2buf")
                col0 = g0 * C * P // 16
                nc.gpsimd.dma_gather(
                    x2buf[:, :gb * C * E].rearrange("p (g e) -> p g e", e=E),
                    cemb[:], idxx2[:, col0:col0 + nx2 // 16], nx2, nx2, E,
                    single_packet=False)
                x2T = [sb_xt.tile([P, GB * P], F32, tag=f"x2T{k}", name=f"x2T{k}")
                       for k in range(KC2)]
                for gg in range(gb):
                    for k in range(KC2):
                        tp = ps_tp.tile([P, P], F32, tag="tp")
                        nc.tensor.transpose(
                            tp[:], x2buf[:, gg * C * E + k * P: gg * C * E + (k + 1) * P],
                            ident[:])
                        if (gg * KC2 + k) % 2 == 0:
                            nc.scalar.copy(x2T[k][:, gg * P:(gg + 1) * P], tp[:])
                        else:
                            nc.vector.tensor_copy(x2T[k][:, gg * P:(gg + 1) * P], tp[:])
                psZ = ps_mm.tile([P, GB * P], F32, tag="mm")
                for k in range(KC2):
                    nc.tensor.matmul(psZ[:, :gb * P], lhsT=wcc[:, k * P:(k + 1) * P],
                                     rhs=x2T[k][:, :gb * P],
                                     start=(k == 0), stop=(k == KC2 - 1))
                s2 = sb_ep.tile([E, GB * P], F32, tag="s")
                nc.scalar.activation(out=s2[:, :gb * P], in_=psZ[0:E, :gb * P],
                                     func=AF.Sigmoid, bias=bcc[:], scale=1.0)
                u = sb_ep.tile([E, GB * P], F32, tag="t")
                nc.vector.scalar_tensor_tensor(
                    out=u[:, :gb * P], in0=psZ[E:2 * E, :gb * P],
                    scalar=bcc2[:], in1=s2[:, :gb * P],
                    op0=ALU.add, op1=ALU.add)
                for gg in range(gb):
                    g = g0 + gg
                    tp2 = ps_sm.tile([P, E], F32, tag="sm")
                    nc.tensor.transpose(tp2[:], u[:, gg * P:(gg + 1) * P],
                                        ident[0:E, 0:E])
                    uraw = sb_ep.tile([P, E], F32, tag="craw")
                    nc.scalar.copy(uraw[:], tp2[:])
                    sq = sb_ep.tile([P, E], F32, tag="sq")
                    ss = sb_ep.tile([P, 1], F32, tag="ss")
                    nc.vector.tensor_mul(sq[:], uraw[:], uraw[:])
                    nc.vector.reduce_sum(ss[:], sq[:], axis=mybir.AxisListType.X)
                    rstd = sb_ep.tile([P, 1], F32, tag="rstd")
                    nc.scalar.activation(out=rstd[:], in_=ss[:], func=AF.Sqrt,
                                         bias=epsv[:])
                    nc.vector.reciprocal(out=rstd[:], in_=rstd[:])
                    nv = nxt[:, g * E:(g + 1) * E]
                    nc.vector.tensor_scalar_mul(nv, uraw[:], rstd[:])
                    # keep old value where the variable has no clause
                    nc.vector.copy_predicated(
                        nv, nocl[:, g:g + 1].broadcast_to([P, E]),
                        cur[:, g * E:(g + 1) * E])
                    if it == 0:
                        # negated embedding for the next iteration's table
                        tpn = ps_sm.tile([E, P], F32, tag="sm")
                        nc.tensor.transpose(tpn[:], nv, ident[:])
                        nvT = sb_ep.tile([E + 1, P], F32, tag="nvT")
                        nc.scalar.copy(nvT[0:E, :], tpn[:])
                        nc.vector.memset(nvT[E:E + 1, :], 1.0)
                        psN = ps_sm.tile([P, E], F32, tag="sm")
                        nc.tensor.matmul(psN[:], lhsT=nvT[:], rhs=wneg[:],
                                         start=True, stop=True)
                        nc.scalar.copy(negstage[:, g * E:(g + 1) * E], psN[:])

        nc.sync.dma_start(t_out[:], shard[0][:])
    nc.compile()
    return nc


def _prep_inputs(plan: HostPlan, inputs, TB=8):
    """Build per-core in_maps + the initial tables."""
    V, G, C, M, E = plan.V, plan.G, plan.C, plan.M, plan.E
    VS, NG = plan.VS, plan.NG
    init = np.concatenate([
        _l2norm(np.asarray(inputs["emb_table"], np.float32)),
        np.tile(_l2norm(np.asarray(inputs["tseitin_emb"], np.float32))[None, :],
                (V - G, 1)),
    ], axis=0)                                        # [V, E]
    W_neg = np.asarray(inputs["W_neg"], np.float32)
    b_neg = np.asarray(inputs["b_neg"], np.float32)
    neg0 = init @ W_neg + b_neg                       # [V, E]
    false_emb = np.asarray(inputs["false_emb"], np.float32)

    wvc = np.concatenate([np.asarray(inputs["vc_W1"], np.float32),
                          np.asarray(inputs["vc_W2"], np.float32)], axis=1)  # [384,128]
    wcc = np.concatenate([np.asarray(inputs["cc_W1"], np.float32),
                          np.asarray(inputs["cc_W2"], np.float32)], axis=1)  # [512,128]
    KC, KC2 = (M * E) // P, (C * E) // P
    wvc_sb = np.concatenate([wvc[k * P:(k + 1) * P] for k in range(KC)], axis=1)
    wcc_sb = np.concatenate([wcc[k * P:(k + 1) * P] for k in range(KC2)], axis=1)
    bvc = np.concatenate([np.asarray(inputs["vc_b1"], np.float32),
                          np.asarray(inputs["vc_b2"], np.float32)])[:, None]
    bcc = np.concatenate([np.asarray(inputs["cc_b1"], np.float32),
                          np.asarray(inputs["cc_b2"], np.float32)])[:, None]
    wneg_aug = np.concatenate([W_neg, b_neg[None, :]], axis=0)  # [65, 64]

    in_maps = []
    for k in range(NCORES):
        vlo = k * VS
        # compact table for iteration 1
        ctab = np.zeros((plan.TC, E), np.float32)
        pl, ng = plan.plain_list[k], plan.neg_list[k]
        j = np.arange(len(pl))
        ctab[(j % P) * plan.GP + j // P] = init[pl]
        j = np.arange(len(ng))
        ctab[plan.NP + (j % P) * plan.GN + j // P] = neg0[ng]
        ctab[plan.FALSE_POS] = false_emb
        # shard in raw sbuf-major layout: row (l%128)*NG + l//128 -> partition
        # l%128, group l//128  => sbuf [128, NG*E]
        sh = np.zeros((plan.VSP, E), np.float32)
        sh[:VS] = init[vlo:vlo + VS]
        shard0 = sh.reshape(NG, P, E).transpose(1, 0, 2).reshape(P, NG * E)
        in_maps.append({
            "compact1": ctab,
            "idx_main": plan.wrap_main_idx(k, TB),
            "idx_prod_p": plan.idx_prod_p[k],
            "idx_prod_n": plan.idx_prod_n[k],
            "idx_x2": plan.idx_x2[k],
            "shard0": shard0,
            "noclause": plan.noclause[k],
            "wvc": wvc_sb, "wcc": wcc_sb,
            "bvc": bvc[:64], "bvc2": bvc[64:], "bcc": bcc[:64], "bcc2": bcc[64:],
            "wneg_aug": wneg_aug,
            "false_row": false_emb[None, :],
        })
    return in_maps


def run(inputs, V, G, C, M, E, trace=False, stages=99):
    plan = HostPlan(V, G, C, M, E,
                    np.asarray(inputs["clause_lits"]),
                    np.asarray(inputs["lit_neg"]),
                    np.asarray(inputs["lit_valid"]),
                    np.asarray(inputs["clause_valid"]))
    nc = build_program(plan, stages=stages)
    in_maps = _prep_inputs(plan, inputs)
    res = run_bass_kernel_spmd(nc, in_maps, core_ids=list(range(NCORES)),
                               trace=trace)
    VS, NG = plan.VS, plan.NG
    out = np.empty((V, E), np.float32)
    for k in range(NCORES):
        raw = res.results[k]["out_shard"]            # [128, NG*E]
        sh = raw.reshape(P, NG, E).transpose(1, 0, 2).reshape(plan.VSP, E)
        out[k * VS:(k + 1) * VS] = sh[:VS]
    return out, res


def kernel(**inputs) -> np.ndarray:
    out, _ = run(inputs, V=20000, G=16000, C=8, M=6, E=64)
    return out

